# revision 7
# baseline (speedup 1.0000x reference)
"""Trainium2 Bass kernel for nn_Destroy: y = (U kron I2) @ x.

The operator reduces to a shift-and-scale over rows:
    y[r, :] = sqrt(r//2 + 1) * x[r+2, :]   for r < 2D-2
    y[2D-2:, :] = 0
with x of shape (2D, B) = (8192, 4096) f32.

Strategy: shard along rows (dim 0), 1024 output rows per core. The +2 row
shift is absorbed into the host-side slice each core receives, so the device
kernel is a pure per-partition scale: 8 tiles of (128, 4096) f32 per core,
DMA in -> tensor_scalar mult (per-partition coefficient) in-place -> DMA out,
triple+ buffered via the Tile framework. Memory-bound: 16 MiB in + 16 MiB out
per core (~94 us at the ~358 GB/s per-core HBM limit).
"""

import sys
import types

import numpy as np

import concourse.bacc as bacc
import concourse.mybir as mybir
import concourse.tile as tile
from concourse import bass_utils


def _ensure_ntff_hook():
    """The axon trace path imports antenv.axon_hooks, which this image's
    antenv package lacks. Provide the tiny get/set module and register the
    ctypes-based NTFF hook from trn_agent_boot so trace=True works."""
    try:
        from antenv import axon_hooks  # noqa: F401
        return
    except ImportError:
        pass
    mod = types.ModuleType("antenv.axon_hooks")
    state = {"hook": None}
    mod.set_axon_ntff_profile_hook = lambda h: state.__setitem__("hook", h)
    mod.get_axon_ntff_profile_hook = lambda: state["hook"]
    sys.modules["antenv.axon_hooks"] = mod
    try:
        import antenv
        antenv.axon_hooks = mod
    except ImportError:
        pass
    try:
        from trn_agent_boot.trn_boot import _ntff_profile_via_ctypes
        mod.set_axon_ntff_profile_hook(
            _ntff_profile_via_ctypes("/opt/axon/libaxon_pjrt.so")
        )
    except Exception:
        pass


_ensure_ntff_hook()

TWO_D = 8192
B = 4096
N_CORES = 8
ROWS = TWO_D // N_CORES  # 1024 output rows per core
P = 128
N_TILES = ROWS // P  # 8

_cached_nc = None
IMPL = "raw"  # "raw" (hand-rolled pipeline, no tile barrier) or "tile"


def _coef_for_core(k: int) -> np.ndarray:
    """coef[p, t] = sqrt(g//2 + 1) for global output row g = 1024*k + 128*t + p,
    zeroed for the last two rows (g >= 2D-2)."""
    g = ROWS * k + np.arange(ROWS)
    c = np.where(g < TWO_D - 2, np.sqrt(g // 2 + 1.0), 0.0).astype(np.float32)
    return np.ascontiguousarray(c.reshape(N_TILES, P).T)  # (P, N_TILES)


def _build_raw():
    """Hand-rolled pipeline: SP queues the coef DMA is on the ACT HWDGE ring;
    all 8 in-DMAs are queued on the SP ring up front (8 dedicated buffers),
    DVE/ACT scale tiles in-place as each lands, and out-DMAs follow FIFO on
    the SP ring gated on the per-tile compute. No Tile drain/barrier tail."""
    import concourse.bass as bass

    nc = bass.Bass("TRN2", debug=False, num_devices=N_CORES)
    f32 = mybir.dt.float32
    x = nc.dram_tensor("x", [ROWS, B], f32, kind="ExternalInput").ap()
    coef = nc.dram_tensor("coef", [P, N_TILES], f32, kind="ExternalInput").ap()
    y = nc.dram_tensor("y", [ROWS, B], f32, kind="ExternalOutput").ap()

    bufs = nc.alloc_sbuf_tensor("bufs", [P, N_TILES, B], f32).ap()
    coef_sb = nc.alloc_sbuf_tensor("coef_sb", [P, N_TILES], f32).ap()

    xt = x.rearrange("(t p) b -> t p b", p=P)
    yt = y.rearrange("(t p) b -> t p b", p=P)

    with nc.Block(no_gpsimd_drain=True) as block:
        # One completion sem per in-DMA: a shared counter races across the 16
        # SDMA engines (per-engine FIFO, cross-engine skew), so 16*(t+1) on a
        # shared sem does NOT imply tile t landed.
        csem = nc.alloc_semaphore("csem")
        in_sems = [nc.alloc_semaphore(f"insem{t}") for t in range(N_TILES)]
        vsem = nc.alloc_semaphore("vsem")
        asem = nc.alloc_semaphore("asem")
        dsem_out = nc.alloc_semaphore("dsem_out")

        @block.sync
        def _(sync: bass.BassEngine):
            for t in range(N_TILES):
                sync.dma_start(out=bufs[:, t], in_=xt[t]).then_inc(in_sems[t], 16)
            for t in range(N_TILES):
                sem, cnt = (vsem, t // 2 + 1) if t % 2 == 0 else (asem, t // 2 + 1)
                sync.wait_ge(sem, cnt)
                sync.dma_start(out=yt[t], in_=bufs[:, t]).then_inc(dsem_out, 16)
            sync.wait_ge(dsem_out, 16 * N_TILES)

        @block.vector
        def _(vector: bass.BassEngine):
            vector.wait_ge(csem, 16)
            for t in range(0, N_TILES, 2):
                vector.wait_ge(in_sems[t], 16)
                vector.tensor_scalar(
                    bufs[:, t], bufs[:, t], coef_sb[:, t : t + 1], None,
                    mybir.AluOpType.mult,
                ).then_inc(vsem, 1)

        @block.scalar
        def _(scalar: bass.BassEngine):
            scalar.dma_start(out=coef_sb[:], in_=coef[:]).then_inc(csem, 16)
            scalar.wait_ge(csem, 16)
            for t in range(1, N_TILES, 2):
                scalar.wait_ge(in_sems[t], 16)
                scalar.activation(
                    bufs[:, t], bufs[:, t], mybir.ActivationFunctionType.Copy,
                    scale=coef_sb[:, t : t + 1],
                ).then_inc(asem, 1)

    return nc


def _build_tile():
    nc = bacc.Bacc("TRN2", debug=False, num_devices=N_CORES)
    f32 = mybir.dt.float32
    x = nc.dram_tensor("x", [ROWS, B], f32, kind="ExternalInput").ap()
    coef = nc.dram_tensor("coef", [P, N_TILES], f32, kind="ExternalInput").ap()
    y = nc.dram_tensor("y", [ROWS, B], f32, kind="ExternalOutput").ap()

    with tile.TileContext(nc) as tc:
        with (
            tc.tile_pool(name="cpool", bufs=1) as cpool,
            tc.tile_pool(name="io", bufs=4) as io,
        ):
            coef_sb = cpool.tile([P, N_TILES], f32)
            nc.sync.dma_start(out=coef_sb[:], in_=coef[:])

            xt = x.rearrange("(t p) b -> t p b", p=P)
            yt = y.rearrange("(t p) b -> t p b", p=P)
            for t in range(N_TILES):
                buf = io.tile([P, B], f32)
                nc.sync.dma_start(out=buf[:], in_=xt[t])
                if t % 2 == 0:
                    nc.vector.tensor_scalar(
                        buf[:], buf[:], coef_sb[:, t : t + 1], None,
                        mybir.AluOpType.mult,
                    )
                else:
                    nc.scalar.activation(
                        buf[:], buf[:], mybir.ActivationFunctionType.Copy,
                        scale=coef_sb[:, t : t + 1],
                    )
                nc.sync.dma_start(out=yt[t], in_=buf[:])

    nc.compile()
    return nc


def _build():
    global _cached_nc
    if _cached_nc is not None:
        return _cached_nc
    _cached_nc = _build_raw() if IMPL == "raw" else _build_tile()
    return _cached_nc


def _shard(x: np.ndarray, k: int) -> np.ndarray:
    """Rows this core reads: global [1024k+2, 1024k+1026), zero-padded past 2D."""
    lo = ROWS * k + 2
    hi = lo + ROWS
    if hi <= TWO_D:
        return x[lo:hi]  # contiguous view, no copy
    pad = np.zeros((ROWS, B), dtype=x.dtype)
    pad[: TWO_D - lo] = x[lo:TWO_D]
    return pad


def run(x: np.ndarray, trace: bool = False):
    assert x.shape == (TWO_D, B), x.shape
    x = np.ascontiguousarray(x, dtype=np.float32)
    nc = _build()
    in_maps = [{"x": _shard(x, k), "coef": _coef_for_core(k)} for k in range(N_CORES)]
    res = bass_utils.run_bass_kernel_spmd(nc, in_maps, list(range(N_CORES)), trace=trace)
    y = np.concatenate([res.results[k]["y"] for k in range(N_CORES)], axis=0)
    return y, res


def kernel(x: np.ndarray) -> np.ndarray:
    y, _ = run(x)
    return y


# revision 10
# speedup vs baseline: 1.1436x; 1.1436x over previous
"""Trainium2 Bass kernel for nn_Destroy: y = (U kron I2) @ x.

The operator reduces to a shift-and-scale over rows:
    y[r, :] = sqrt(r//2 + 1) * x[r+2, :]   for r < 2D-2
    y[2D-2:, :] = 0
with x of shape (2D, B) = (8192, 4096) f32.

Strategy: shard along rows (dim 0), 1024 output rows per core. The +2 row
shift is absorbed into the host-side slice each core receives, so the device
kernel is a pure per-partition scale: 8 tiles of (128, 4096) f32 per core,
DMA in -> tensor_scalar mult (per-partition coefficient) in-place -> DMA out,
triple+ buffered via the Tile framework. Memory-bound: 16 MiB in + 16 MiB out
per core (~94 us at the ~358 GB/s per-core HBM limit).
"""

import sys
import types

import numpy as np

import concourse.bacc as bacc
import concourse.mybir as mybir
import concourse.tile as tile
from concourse import bass_utils


def _ensure_ntff_hook():
    """The axon trace path imports antenv.axon_hooks, which this image's
    antenv package lacks. Provide the tiny get/set module and register the
    ctypes-based NTFF hook from trn_agent_boot so trace=True works."""
    try:
        from antenv import axon_hooks  # noqa: F401
        return
    except ImportError:
        pass
    mod = types.ModuleType("antenv.axon_hooks")
    state = {"hook": None}
    mod.set_axon_ntff_profile_hook = lambda h: state.__setitem__("hook", h)
    mod.get_axon_ntff_profile_hook = lambda: state["hook"]
    sys.modules["antenv.axon_hooks"] = mod
    try:
        import antenv
        antenv.axon_hooks = mod
    except ImportError:
        pass
    try:
        from trn_agent_boot.trn_boot import _ntff_profile_via_ctypes
        mod.set_axon_ntff_profile_hook(
            _ntff_profile_via_ctypes("/opt/axon/libaxon_pjrt.so")
        )
    except Exception:
        pass


_ensure_ntff_hook()

TWO_D = 8192
B = 4096
N_CORES = 8
ROWS = TWO_D // N_CORES  # 1024 output rows per core
P = 128
N_TILES = ROWS // P  # 8

_cached_nc = None
IMPL = "raw"  # "raw" (hand-rolled pipeline, no tile barrier) or "tile"


def _coef_for_core(k: int) -> np.ndarray:
    """coef[p, t] = sqrt(g//2 + 1) for global output row g = 1024*k + 128*t + p,
    zeroed for the last two rows (g >= 2D-2)."""
    g = ROWS * k + np.arange(ROWS)
    c = np.where(g < TWO_D - 2, np.sqrt(g // 2 + 1.0), 0.0).astype(np.float32)
    return np.ascontiguousarray(c.reshape(N_TILES, P).T)  # (P, N_TILES)


def _build_raw():
    """Hand-rolled pipeline: SP queues the coef DMA is on the ACT HWDGE ring;
    all 8 in-DMAs are queued on the SP ring up front (8 dedicated buffers),
    DVE/ACT scale tiles in-place as each lands, and out-DMAs follow FIFO on
    the SP ring gated on the per-tile compute. No Tile drain/barrier tail."""
    import concourse.bass as bass

    nc = bass.Bass("TRN2", debug=False, num_devices=N_CORES)
    f32 = mybir.dt.float32
    x = nc.dram_tensor("x", [ROWS, B], f32, kind="ExternalInput").ap()
    coef = nc.dram_tensor("coef", [P, N_TILES], f32, kind="ExternalInput").ap()
    y = nc.dram_tensor("y", [ROWS, B], f32, kind="ExternalOutput").ap()

    bufs = nc.alloc_sbuf_tensor("bufs", [P, N_TILES, B], f32).ap()
    coef_sb = nc.alloc_sbuf_tensor("coef_sb", [P, N_TILES], f32).ap()

    xt = x.rearrange("(t p) b -> t p b", p=P)
    yt = y.rearrange("(t p) b -> t p b", p=P)

    # One completion sem per in-DMA: a shared counter races across the 16
    # SDMA engines (per-engine FIFO, cross-engine skew), so 16*(t+1) on a
    # shared sem does NOT imply tile t landed.
    csem = nc.alloc_semaphore("csem")
    in_sems = [nc.alloc_semaphore(f"insem{t}") for t in range(N_TILES)]
    vsem = nc.alloc_semaphore("vsem")
    asem = nc.alloc_semaphore("asem")
    dsem_out = nc.alloc_semaphore("dsem_out")

    # Block-body structure without Block's exit barrier: every cross-engine
    # dependency is already enforced by the sems above, and SP's final wait
    # holds the program open until the last output byte lands -- the ~7us
    # all-engine EVSEM barrier at block exit adds nothing here.
    block = bass.BassBlock(nc, f"blk_{nc.next_id()}")
    nc.cur_block = block
    try:

        @block.sync
        def _(sync: bass.BassEngine):
            for t in range(N_TILES):
                sync.dma_start(out=bufs[:, t], in_=xt[t]).then_inc(in_sems[t], 16)
            for t in range(N_TILES):
                sem, cnt = (vsem, t // 2 + 1) if t % 2 == 0 else (asem, t // 2 + 1)
                sync.wait_ge(sem, cnt)
                sync.dma_start(out=yt[t], in_=bufs[:, t]).then_inc(dsem_out, 16)
            sync.wait_ge(dsem_out, 16 * N_TILES)

        @block.vector
        def _(vector: bass.BassEngine):
            vector.wait_ge(csem, 16)
            for t in range(0, N_TILES, 2):
                vector.wait_ge(in_sems[t], 16)
                vector.tensor_scalar(
                    bufs[:, t], bufs[:, t], coef_sb[:, t : t + 1], None,
                    mybir.AluOpType.mult,
                ).then_inc(vsem, 1)

        @block.scalar
        def _(scalar: bass.BassEngine):
            scalar.dma_start(out=coef_sb[:], in_=coef[:]).then_inc(csem, 16)
            scalar.wait_ge(csem, 16)
            for t in range(1, N_TILES, 2):
                scalar.wait_ge(in_sems[t], 16)
                scalar.activation(
                    bufs[:, t], bufs[:, t], mybir.ActivationFunctionType.Copy,
                    scale=coef_sb[:, t : t + 1],
                ).then_inc(asem, 1)

        for engine, last_body in block.last_body.items():
            with nc.body(last_body, parent=nc.cur_bb, allow_existing_parent=True):
                engine.br(block.end_bb)
        nc.switch_bb(block.end_bb)
    finally:
        nc.cur_block = None

    # Strip the Bass-preamble all-engine barrier (Drain + EventSemaphore per
    # engine) and the const-AP memsets from the entry block: this kernel uses
    # no const_aps and every cross-engine ordering is enforced by explicit
    # semaphores, so the ~7us startup barrier only delays the first DMA.
    entry = nc.m.functions[0].blocks[0]
    entry.instructions[:] = [
        i for i in entry.instructions
        if not (
            isinstance(i, (mybir.InstMemset, mybir.InstDrain))
            or (isinstance(i, mybir.InstEventSemaphore)
                and i.name.startswith("barrier_"))
        )
    ]

    return nc


def _build_tile():
    nc = bacc.Bacc("TRN2", debug=False, num_devices=N_CORES)
    f32 = mybir.dt.float32
    x = nc.dram_tensor("x", [ROWS, B], f32, kind="ExternalInput").ap()
    coef = nc.dram_tensor("coef", [P, N_TILES], f32, kind="ExternalInput").ap()
    y = nc.dram_tensor("y", [ROWS, B], f32, kind="ExternalOutput").ap()

    with tile.TileContext(nc) as tc:
        with (
            tc.tile_pool(name="cpool", bufs=1) as cpool,
            tc.tile_pool(name="io", bufs=4) as io,
        ):
            coef_sb = cpool.tile([P, N_TILES], f32)
            nc.sync.dma_start(out=coef_sb[:], in_=coef[:])

            xt = x.rearrange("(t p) b -> t p b", p=P)
            yt = y.rearrange("(t p) b -> t p b", p=P)
            for t in range(N_TILES):
                buf = io.tile([P, B], f32)
                nc.sync.dma_start(out=buf[:], in_=xt[t])
                if t % 2 == 0:
                    nc.vector.tensor_scalar(
                        buf[:], buf[:], coef_sb[:, t : t + 1], None,
                        mybir.AluOpType.mult,
                    )
                else:
                    nc.scalar.activation(
                        buf[:], buf[:], mybir.ActivationFunctionType.Copy,
                        scale=coef_sb[:, t : t + 1],
                    )
                nc.sync.dma_start(out=yt[t], in_=buf[:])

    nc.compile()
    return nc


def _build():
    global _cached_nc
    if _cached_nc is not None:
        return _cached_nc
    _cached_nc = _build_raw() if IMPL == "raw" else _build_tile()
    return _cached_nc


def _shard(x: np.ndarray, k: int) -> np.ndarray:
    """Rows this core reads: global [1024k+2, 1024k+1026), zero-padded past 2D."""
    lo = ROWS * k + 2
    hi = lo + ROWS
    if hi <= TWO_D:
        return x[lo:hi]  # contiguous view, no copy
    pad = np.zeros((ROWS, B), dtype=x.dtype)
    pad[: TWO_D - lo] = x[lo:TWO_D]
    return pad


def run(x: np.ndarray, trace: bool = False):
    assert x.shape == (TWO_D, B), x.shape
    x = np.ascontiguousarray(x, dtype=np.float32)
    nc = _build()
    in_maps = [{"x": _shard(x, k), "coef": _coef_for_core(k)} for k in range(N_CORES)]
    res = bass_utils.run_bass_kernel_spmd(nc, in_maps, list(range(N_CORES)), trace=trace)
    y = np.concatenate([res.results[k]["y"] for k in range(N_CORES)], axis=0)
    return y, res


def kernel(x: np.ndarray) -> np.ndarray:
    y, _ = run(x)
    return y
